# revision 6
# baseline (speedup 1.0000x reference)
"""Contrastive loss (NT-Xent style) Trainium2 kernel, 8-core SPMD.

Math: with z_i = normalize(instance_emb.reshape(4096, 512)),
zbag = normalize(bag_emb) [8, 512], z_j = repeat(zbag, 512) and
Z = [z_i; z_j] (8192 rows), the reference computes

  loss = (1/8192) * sum_r [ log(sum_{c != r} exp(2*sim[r,c])) - 2*pos[r] ]

with sim = Z @ Z.T, pos[r] = sim[r, r +- 4096].  Because the z_j half
consists of only 8 distinct rows (each repeated 512x), only the
G = z_i @ z_i.T quadrant (4096x4096) needs dense compute:

  denom_i[r] = rowsum(exp(2 G[r,:])) - e^2 + 512 * sum_g exp(2 S1[r,g])
  denom_j[g] = colsum_r(exp(2 S1[r,g])) + 512 * rowsum(exp(2 B[g,:])) - e^2
  pos[r] = pos[bs + r] = S1[r, r // 512]
  loss*8192 = sum_r [log denom_i[r] - 4*S1[r, r//512]] + 512*sum_g log denom_j[g]

where S1 = z_i @ zbag.T [4096, 8] and B = zbag @ zbag.T [8, 8].
(sim[r,r] = ||z_r||^2 = 1 to fp32 roundoff, so the excluded diagonal
term is exp(2) = e^2 up to ~1e-6 relative, i.e. ~2e-9 of the denominator.)

Sharding: 512 G-rows per core.  Each core normalizes + transposes its own
512 rows (PE transpose), AllGathers the transposed block (ZT, [512, 4096]
total), then computes its row-block of G with fused exp+rowsum on the
scalar engine (accum_out).  Output: per-core scalar partials; host sums.
"""

import os
import numpy as np
from contextlib import ExitStack

import concourse.bass as bass
import concourse.bacc as bacc
import concourse.tile as tile
from concourse import mybir
from concourse import bass_utils
from concourse.masks import make_identity

F32 = mybir.dt.float32

B, N, D = 8, 512, 512
BS = B * N              # 4096 instance rows
NCORES = 8
RPC = BS // NCORES      # 512 rows per core
TPC = RPC // 128        # 4 row-tiles of 128 per core
KC = D // 128           # 4 contraction chunks
NBLK = BS // 512        # 8 column blocks of 512
E2 = float(np.exp(2.0))

LAST_EXEC_TIME_NS = None
_CACHED_NC = None


def _normalize_rows(nc, pools, x_tiles, ss, ntiles, nparts):
    """x_tiles: list of SBUF tiles [nparts, 512] (raw rows). ss: [nparts, ntiles]
    scratch accumulator tile. Returns r [nparts, ntiles] = 1/||row|| (2 Newton
    steps on top of exp(-0.5*ln(sumsq)))."""
    work = pools["work"]
    sq = work.tile([nparts, D], F32, name=f"sq_scratch_{nparts}")
    for t in range(ntiles):
        nc.vector.tensor_mul(sq, x_tiles[t], x_tiles[t])
        nc.vector.reduce_sum(ss[:, t : t + 1], sq, axis=mybir.AxisListType.X)
    lnss = work.tile([nparts, ntiles], F32, name=f"lnss_{nparts}")
    nc.scalar.activation(lnss, ss, mybir.ActivationFunctionType.Ln)
    r = work.tile([nparts, ntiles], F32, name=f"rnorm_{nparts}")
    nc.scalar.activation(r, lnss, mybir.ActivationFunctionType.Exp, scale=-0.5)
    # Newton: r <- r * (1.5 - 0.5 * ss * r^2), twice (ACT table error insurance)
    a = work.tile([nparts, ntiles], F32, name=f"nta_{nparts}")
    for _ in range(2):
        nc.vector.tensor_mul(a, r, r)
        nc.vector.tensor_mul(a, a, ss)
        nc.vector.tensor_scalar(
            out=a,
            in0=a,
            scalar1=-0.5,
            scalar2=1.5,
            op0=mybir.AluOpType.mult,
            op1=mybir.AluOpType.add,
        )
        nc.vector.tensor_mul(r, r, a)
    return r


def _build_kernel(nc):
    y_own = nc.dram_tensor("y_own", [RPC, D], F32, kind="ExternalInput")
    bag = nc.dram_tensor("bag", [B, D], F32, kind="ExternalInput")
    onehot = nc.dram_tensor("onehot", [1, B], F32, kind="ExternalInput")
    out_d = nc.dram_tensor("out", [1, 2], F32, kind="ExternalOutput")

    with ExitStack() as ctx:
        tc = ctx.enter_context(tile.TileContext(nc))

        consts = ctx.enter_context(tc.tile_pool(name="consts", bufs=1))
        work = ctx.enter_context(tc.tile_pool(name="work", bufs=2))
        persist = ctx.enter_context(tc.tile_pool(name="persist", bufs=1))
        dram = ctx.enter_context(tc.tile_pool(name="dram", bufs=1, space="DRAM"))
        ps_main = ctx.enter_context(tc.tile_pool(name="ps_main", bufs=3, space="PSUM"))
        ps_tr = ctx.enter_context(tc.tile_pool(name="ps_tr", bufs=2, space="PSUM"))
        ps_sm = ctx.enter_context(tc.tile_pool(name="ps_sm", bufs=3, space="PSUM"))
        pools = {"work": work}

        ident = consts.tile([128, 128], F32)
        make_identity(nc, ident)
        ones = consts.tile([128, 1], F32)
        nc.vector.memset(ones, 1.0)
        oh = consts.tile([128, B], F32)
        nc.sync.dma_start(out=oh, in_=onehot.ap().to_broadcast((128, B)))

        # ---- load + normalize own rows ----
        y_tiles = []
        for t in range(TPC):
            yt = persist.tile([128, D], F32, name=f"y_{t}")
            nc.sync.dma_start(out=yt, in_=y_own[t * 128 : (t + 1) * 128, :])
            y_tiles.append(yt)
        ss = persist.tile([128, TPC], F32, name="ss_own")
        r_own = _normalize_rows(nc, pools, y_tiles, ss, TPC, 128)

        # ---- load + normalize bag ----
        bag_t = persist.tile([B, D], F32, name="bag_t")
        nc.sync.dma_start(out=bag_t, in_=bag[:, :])
        ss_b = persist.tile([B, 1], F32, name="ss_bag")
        r_bag = _normalize_rows(nc, pools, [bag_t], ss_b, 1, B)
        zbag = persist.tile([B, D], F32, name="zbag")
        nc.vector.tensor_scalar_mul(zbag, bag_t, r_bag[:, 0:1])

        # zbagT chunks: [128, B] per K chunk, as one [128, KC, B] tile
        zbagT = persist.tile([128, KC, B], F32, name="zbagT")
        for k in range(KC):
            ptr = ps_tr.tile([128, B], F32, tag="ptr", name="ptr_bag")
            nc.tensor.transpose(ptr, zbag[:, k * 128 : (k + 1) * 128], ident[:B, :B])
            nc.vector.tensor_copy(zbagT[:, k, :], ptr)

        # ---- scale + transpose own rows -> ztown[k] = [128(d), RPC] ----
        ztown = [persist.tile([128, RPC], F32, name=f"ztown_{k}") for k in range(KC)]
        for t in range(TPC):
            zt = work.tile([128, D], F32, name="z_t")
            nc.vector.tensor_scalar_mul(zt, y_tiles[t], r_own[:, t : t + 1])
            for k in range(KC):
                ptr = ps_tr.tile([128, 128], F32, tag="ptr", name="ptr_own")
                nc.tensor.transpose(ptr, zt[:, k * 128 : (k + 1) * 128], ident)
                nc.vector.tensor_copy(
                    ztown[k][:, t * 128 : (t + 1) * 128], ptr
                )

        # ---- AllGather ZT ----
        ag_in = dram.tile([D, RPC], F32, name="ag_in")
        ag_out = dram.tile([NCORES * D, RPC], F32, name="ag_out", addr_space="Shared")
        for k in range(KC):
            nc.sync.dma_start(out=ag_in[k * 128 : (k + 1) * 128, :], in_=ztown[k])
        nc.gpsimd.collective_compute(
            "AllGather",
            mybir.AluOpType.bypass,
            replica_groups=[list(range(NCORES))],
            ins=[ag_in.opt()],
            outs=[ag_out.opt()],
        )

        # ---- load full ZT: ztf[b][k] = [128, 512] (cols = rank b's rows) ----
        ztf = {}
        for b in range(NBLK):
            for k in range(KC):
                tl = persist.tile([128, 512], F32, name=f"ztf_{b}_{k}")
                nc.sync.dma_start(
                    out=tl, in_=ag_out[b * D + k * 128 : b * D + (k + 1) * 128, :]
                )
                ztf[(b, k)] = tl

        # ---- accumulators ----
        rs = persist.tile([128, TPC, NBLK], F32, name="rs")       # rowsum exp(2G)
        s1rs = persist.tile([128, TPC], F32, name="s1rs")         # rowsum exp(2 S1own)
        pos = persist.tile([128, TPC], F32, name="pos")           # own positives
        cs = persist.tile([B, NBLK], F32, name="cs")              # colsum exp(2 S1) per blk
        s1sc = persist.tile([128, B], F32, name="s1sc")           # scratch for ttr product

        # ---- S1 own rows + positives (PE warm-up work during AG) ----
        for t in range(TPC):
            pm = ps_sm.tile([128, B], F32, tag="sm", name="ps_s1own")
            for k in range(KC):
                nc.tensor.matmul(
                    pm,
                    lhsT=ztown[k][:, t * 128 : (t + 1) * 128],
                    rhs=zbagT[:, k, :],
                    start=(k == 0),
                    stop=(k == KC - 1),
                )
            es = work.tile([128, B], F32, name="exp_s1own")
            nc.scalar.activation(
                es, pm, mybir.ActivationFunctionType.Exp, scale=2.0,
                accum_out=s1rs[:, t : t + 1],
            )
            nc.vector.tensor_mul(s1sc, pm, oh)
            nc.vector.reduce_sum(
                pos[:, t : t + 1], s1sc, axis=mybir.AxisListType.X
            )

        # ---- Bgram ----
        pbg = ps_sm.tile([B, B], F32, tag="sm", name="ps_bgram")
        for k in range(KC):
            nc.tensor.matmul(
                pbg, lhsT=zbagT[:, k, :], rhs=zbagT[:, k, :],
                start=(k == 0), stop=(k == KC - 1),
            )
        ebg = persist.tile([B, B], F32, name="exp_bgram")
        rsbg = persist.tile([B, 1], F32, name="rs_bgram")
        nc.scalar.activation(
            ebg, pbg, mybir.ActivationFunctionType.Exp, scale=2.0, accum_out=rsbg
        )

        # ---- main loop: G row-block x col-block, and S1T per col-block ----
        for blk in range(NBLK):
            for t in range(TPC):
                pm = ps_main.tile([128, 512], F32, name="ps_g")
                for k in range(KC):
                    nc.tensor.matmul(
                        pm,
                        lhsT=ztown[k][:, t * 128 : (t + 1) * 128],
                        rhs=ztf[(blk, k)],
                        start=(k == 0),
                        stop=(k == KC - 1),
                    )
                # exp(2*G) in place in PSUM; rowsum via accum
                nc.scalar.activation(
                    pm, pm, mybir.ActivationFunctionType.Exp, scale=2.0,
                    accum_out=rs[:, t, blk : blk + 1],
                )
            # S1T row for this block: [B, 512]
            pst = ps_sm.tile([B, 512], F32, tag="sm", name="ps_s1t")
            for k in range(KC):
                nc.tensor.matmul(
                    pst, lhsT=zbagT[:, k, :], rhs=ztf[(blk, k)],
                    start=(k == 0), stop=(k == KC - 1),
                )
            nc.scalar.activation(
                pst, pst, mybir.ActivationFunctionType.Exp, scale=2.0,
                accum_out=cs[:, blk : blk + 1],
            )

        # ---- denominators + logs ----
        # denom_i [128, TPC]
        rsum = persist.tile([128, TPC], F32, name="rsum")
        nc.vector.reduce_sum(rsum, rs, axis=mybir.AxisListType.X)
        di = persist.tile([128, TPC], F32, name="di")
        nc.vector.tensor_scalar(
            out=di, in0=s1rs, scalar1=512.0, scalar2=-E2,
            op0=mybir.AluOpType.mult, op1=mybir.AluOpType.add,
        )
        nc.vector.tensor_add(di, di, rsum)
        ldi = persist.tile([128, TPC], F32, name="ldi")
        nc.scalar.activation(ldi, di, mybir.ActivationFunctionType.Ln)

        # denom_j [B, 1]
        csum = persist.tile([B, 1], F32, name="csum")
        nc.vector.reduce_sum(csum, cs, axis=mybir.AxisListType.X)
        dj = persist.tile([B, 1], F32, name="dj")
        nc.vector.tensor_scalar(
            out=dj, in0=rsbg, scalar1=512.0, scalar2=-E2,
            op0=mybir.AluOpType.mult, op1=mybir.AluOpType.add,
        )
        nc.vector.tensor_add(dj, dj, csum)
        ldj = persist.tile([B, 1], F32, name="ldj")
        nc.scalar.activation(ldj, dj, mybir.ActivationFunctionType.Ln)

        # ---- final combine: fin[:,0] = sum_t ldi - 4*sum_t pos; fin[0:8,1] = 512*ldj
        fin = persist.tile([128, 2], F32, name="fin")
        nc.vector.memset(fin, 0.0)
        vsum = persist.tile([128, 1], F32, name="vsum")
        nc.vector.reduce_sum(vsum, ldi, axis=mybir.AxisListType.X)
        posr = persist.tile([128, 1], F32, name="posr")
        nc.vector.reduce_sum(posr, pos, axis=mybir.AxisListType.X)
        nc.vector.tensor_scalar(
            out=posr, in0=posr, scalar1=-4.0, scalar2=None,
            op0=mybir.AluOpType.mult,
        )
        nc.vector.tensor_add(fin[:, 0:1], vsum, posr)
        nc.scalar.mul(fin[0:B, 1:2], ldj, 512.0)

        pfin = ps_sm.tile([1, 2], F32, tag="sm", name="ps_fin")
        nc.tensor.matmul(pfin, lhsT=ones, rhs=fin, start=True, stop=True)
        outt = persist.tile([1, 2], F32, name="outt")
        nc.vector.tensor_copy(outt, pfin)
        nc.sync.dma_start(out=out_d[:, :], in_=outt)

    return nc


def _get_nc():
    global _CACHED_NC
    if _CACHED_NC is None:
        nc = bacc.Bacc(
            "TRN2", target_bir_lowering=False, debug=False, num_devices=NCORES
        )
        nc = _build_kernel(nc)
        nc.compile()
        _CACHED_NC = nc
    return _CACHED_NC


def kernel(instance_emb: np.ndarray, bag_emb: np.ndarray) -> np.ndarray:
    global LAST_EXEC_TIME_NS
    Y = np.ascontiguousarray(
        np.asarray(instance_emb, dtype=np.float32).reshape(BS, D)
    )
    bg = np.ascontiguousarray(np.asarray(bag_emb, dtype=np.float32))

    in_maps = []
    for c in range(NCORES):
        oh = np.zeros((1, B), np.float32)
        oh[0, c] = 1.0
        in_maps.append(
            {
                "y_own": np.ascontiguousarray(Y[c * RPC : (c + 1) * RPC]),
                "bag": bg,
                "onehot": oh,
            }
        )

    nc = _get_nc()
    trace = os.environ.get("CL_KERNEL_TRACE", "0") == "1"
    tmpdir = os.environ.get("CL_KERNEL_TRACE_DIR") or None
    res = bass_utils.run_bass_kernel_spmd(
        nc, in_maps, core_ids=list(range(NCORES)), trace=trace, tmpdir=tmpdir
    )
    LAST_EXEC_TIME_NS = res.exec_time_ns

    total = 0.0
    for c in range(NCORES):
        total += float(res.results[c]["out"][0, 0])
    total += float(res.results[0]["out"][0, 1])
    return np.float32(total / (2 * BS))


# revision 16
# speedup vs baseline: 2.0705x; 2.0705x over previous
"""Contrastive loss (NT-Xent style) Trainium2 kernel, 8-core SPMD.

Math: with z_i = normalize(instance_emb.reshape(4096, 512)),
zbag = normalize(bag_emb) [8, 512], z_j = repeat(zbag, 512) and
Z = [z_i; z_j] (8192 rows), the reference computes

  loss = (1/8192) * sum_r [ log(sum_{c != r} exp(2*sim[r,c])) - 2*pos[r] ]

with sim = Z @ Z.T, pos[r] = sim[r, r +- 4096].  Because the z_j half
consists of only 8 distinct rows (each repeated 512x), only the
G = z_i @ z_i.T quadrant (4096x4096) needs dense compute:

  denom_i[r] = rowsum(exp(2 G[r,:])) - e^2 + 512 * sum_g exp(2 S1[r,g])
  denom_j[g] = colsum_r(exp(2 S1[r,g])) + 512 * rowsum(exp(2 B[g,:])) - e^2
  pos[r] = pos[bs + r] = S1[r, r // 512]
  loss*8192 = sum_r [log denom_i[r] - 4*S1[r, r//512]] + 512*sum_g log denom_j[g]

where S1 = z_i @ zbag.T [4096, 8] and B = zbag @ zbag.T [8, 8].
(sim[r,r] = ||z_r||^2 = 1 to fp32 roundoff, so the excluded diagonal
term is exp(2) = e^2 up to ~1e-6 relative, i.e. ~2e-9 of the denominator.)

Sharding: 512 G-rows per core.  Each core normalizes + transposes its own
512 rows (PE transpose), AllGathers the transposed block (ZT, [512, 4096]
total), then computes its row-block of G with fused exp+rowsum on the
scalar engine (accum_out).  Output: per-core scalar partials; host sums.
"""

import os
import numpy as np
from contextlib import ExitStack

import concourse.bass as bass
import concourse.bacc as bacc
import concourse.tile as tile
from concourse import mybir
from concourse import bass_utils
from concourse.masks import make_identity

F32 = mybir.dt.float32
BF16 = mybir.dt.bfloat16

B, N, D = 8, 512, 512
BS = B * N              # 4096 instance rows
NCORES = 8
RPC = BS // NCORES      # 512 rows per core
TPC = RPC // 128        # 4 row-tiles of 128 per core
KC = D // 128           # 4 contraction chunks
NBLK = BS // 512        # 8 column blocks of 512
E2 = float(np.exp(2.0))

LAST_EXEC_TIME_NS = None
_CACHED_NC = None


def _normalize_rows(nc, pools, x_tiles, ss, ntiles, nparts):
    """x_tiles: list of SBUF tiles [nparts, 512] (raw rows). ss: [nparts, ntiles]
    scratch accumulator tile. Returns r [nparts, ntiles] = 1/||row|| (2 Newton
    steps on top of exp(-0.5*ln(sumsq)))."""
    work = pools["work"]
    sq = work.tile([nparts, D], F32, name=f"sq_scratch_{nparts}")
    for t in range(ntiles):
        nc.vector.tensor_mul(sq, x_tiles[t], x_tiles[t])
        nc.vector.reduce_sum(ss[:, t : t + 1], sq, axis=mybir.AxisListType.X)
    lnss = work.tile([nparts, ntiles], F32, name=f"lnss_{nparts}")
    nc.scalar.activation(lnss, ss, mybir.ActivationFunctionType.Ln)
    r = work.tile([nparts, ntiles], F32, name=f"rnorm_{nparts}")
    nc.scalar.activation(r, lnss, mybir.ActivationFunctionType.Exp, scale=-0.5)
    # Newton: r <- r * (1.5 - 0.5 * ss * r^2), twice (ACT table error insurance)
    a = work.tile([nparts, ntiles], F32, name=f"nta_{nparts}")
    for _ in range(2):
        nc.vector.tensor_mul(a, r, r)
        nc.vector.tensor_mul(a, a, ss)
        nc.vector.tensor_scalar(
            out=a,
            in0=a,
            scalar1=-0.5,
            scalar2=1.5,
            op0=mybir.AluOpType.mult,
            op1=mybir.AluOpType.add,
        )
        nc.vector.tensor_mul(r, r, a)
    return r


def _build_kernel(nc):
    y_own = nc.dram_tensor("y_own", [RPC, D], F32, kind="ExternalInput")
    bag = nc.dram_tensor("bag", [B, D], F32, kind="ExternalInput")
    onehot = nc.dram_tensor("onehot", [1, B], F32, kind="ExternalInput")
    out_d = nc.dram_tensor("out", [1, 2], F32, kind="ExternalOutput")

    with ExitStack() as ctx:
        tc = ctx.enter_context(tile.TileContext(nc))

        consts = ctx.enter_context(tc.tile_pool(name="consts", bufs=1))
        work = ctx.enter_context(tc.tile_pool(name="work", bufs=2))
        persist = ctx.enter_context(tc.tile_pool(name="persist", bufs=1))
        dram = ctx.enter_context(tc.tile_pool(name="dram", bufs=1, space="DRAM"))
        ps_main = ctx.enter_context(tc.tile_pool(name="ps_main", bufs=3, space="PSUM"))
        ps_sm = ctx.enter_context(tc.tile_pool(name="ps_sm", bufs=2, space="PSUM"))
        ps_tr = ps_sm
        pools = {"work": work}

        ident = consts.tile([128, 128], F32)
        make_identity(nc, ident)
        ones = consts.tile([128, 1], F32)
        nc.vector.memset(ones, 1.0)
        oh = consts.tile([128, B], F32)
        nc.sync.dma_start(out=oh, in_=onehot.ap().to_broadcast((128, B)))

        # ---- load + normalize own rows ----
        y_tiles = []
        for t in range(TPC):
            yt = persist.tile([128, D], F32, name=f"y_{t}")
            nc.sync.dma_start(out=yt, in_=y_own[t * 128 : (t + 1) * 128, :])
            y_tiles.append(yt)
        ss = persist.tile([128, TPC], F32, name="ss_own")
        r_own = _normalize_rows(nc, pools, y_tiles, ss, TPC, 128)

        # ---- load + normalize bag ----
        bag_t = persist.tile([B, D], F32, name="bag_t")
        nc.sync.dma_start(out=bag_t, in_=bag[:, :])
        ss_b = persist.tile([B, 1], F32, name="ss_bag")
        r_bag = _normalize_rows(nc, pools, [bag_t], ss_b, 1, B)
        zbag = persist.tile([B, D], F32, name="zbag")
        nc.vector.tensor_scalar_mul(zbag, bag_t, r_bag[:, 0:1])

        # zbagT chunks: [128, B] per K chunk, as one [128, KC, B] tile
        zbagT = persist.tile([128, KC, B], BF16, name="zbagT")
        for k in range(KC):
            ptr = ps_tr.tile([128, B], F32, tag="sm", name="ptr_bag")
            nc.tensor.transpose(ptr, zbag[:, k * 128 : (k + 1) * 128], ident[:B, :B])
            nc.vector.tensor_copy(zbagT[:, k, :], ptr)

        # ---- scale + transpose own rows -> ztown[k] = [128(d), RPC] ----
        ztown = [persist.tile([128, RPC], BF16, name=f"ztown_{k}") for k in range(KC)]
        for t in range(TPC):
            zt = work.tile([128, D], F32, name="z_t")
            nc.vector.tensor_scalar_mul(zt, y_tiles[t], r_own[:, t : t + 1])
            for k in range(KC):
                ptr = ps_tr.tile([128, 128], F32, tag="sm", name="ptr_own")
                nc.tensor.transpose(ptr, zt[:, k * 128 : (k + 1) * 128], ident)
                nc.vector.tensor_copy(
                    ztown[k][:, t * 128 : (t + 1) * 128], ptr
                )

        # ---- AllGather ZT ----
        ag_in = dram.tile([D, RPC], BF16, name="ag_in")
        ag_out = dram.tile([NCORES * D, RPC], BF16, name="ag_out", addr_space="Shared")
        for k in range(KC):
            nc.sync.dma_start(out=ag_in[k * 128 : (k + 1) * 128, :], in_=ztown[k])
        nc.gpsimd.collective_compute(
            "AllGather",
            mybir.AluOpType.bypass,
            replica_groups=[list(range(NCORES))],
            ins=[ag_in.opt()],
            outs=[ag_out.opt()],
        )

        # ---- load full ZT: ztf[b][k] = [128, 512] (cols = rank b's rows) ----
        ztf = {}
        for b in range(NBLK):
            for k in range(KC):
                tl = persist.tile([128, 512], BF16, name=f"ztf_{b}_{k}")
                nc.sync.dma_start(
                    out=tl, in_=ag_out[b * D + k * 128 : b * D + (k + 1) * 128, :]
                )
                ztf[(b, k)] = tl

        # ---- accumulators ----
        rs = persist.tile([128, TPC, NBLK // 2], F32, name="rs")  # rowsum exp(2G)
        s1rs = persist.tile([128, TPC], F32, name="s1rs")         # rowsum exp(2 S1own)
        pos = persist.tile([128, TPC], F32, name="pos")           # own positives
        cs = persist.tile([B, NBLK], F32, name="cs")              # colsum exp(2 S1) per blk
        s1sc = persist.tile([128, B], F32, name="s1sc")           # scratch for ttr product

        # ---- S1 own rows + positives (PE warm-up work during AG) ----
        for t in range(TPC):
            pm = ps_sm.tile([128, B], F32, tag="sm", name="ps_s1own")
            for k in range(KC):
                nc.tensor.matmul(
                    pm,
                    lhsT=ztown[k][:, t * 128 : (t + 1) * 128],
                    rhs=zbagT[:, k, :],
                    start=(k == 0),
                    stop=(k == KC - 1),
                )
            es = work.tile([128, B], F32, name="exp_s1own")
            nc.scalar.activation(
                es, pm, mybir.ActivationFunctionType.Exp, scale=2.0,
                accum_out=s1rs[:, t : t + 1],
            )
            nc.vector.tensor_mul(s1sc, pm, oh)
            nc.vector.reduce_sum(
                pos[:, t : t + 1], s1sc, axis=mybir.AxisListType.X
            )

        # ---- Bgram ----
        pbg = ps_sm.tile([B, B], F32, tag="sm", name="ps_bgram")
        for k in range(KC):
            nc.tensor.matmul(
                pbg, lhsT=zbagT[:, k, :], rhs=zbagT[:, k, :],
                start=(k == 0), stop=(k == KC - 1),
            )
        ebg = persist.tile([B, B], F32, name="exp_bgram")
        rsbg = persist.tile([B, 1], F32, name="rs_bgram")
        nc.scalar.activation(
            ebg, pbg, mybir.ActivationFunctionType.Exp, scale=2.0, accum_out=rsbg
        )

        # ---- main loop: G row-block x double col-block, S1T per col-block ----
        for bb in range(NBLK // 2):
            for t in range(TPC):
                pm = ps_main.tile([128, 1024], F32, name="ps_g")
                for half in range(2):
                    blk = 2 * bb + half
                    for k in range(KC):
                        nc.tensor.matmul(
                            pm[:, half * 512 : (half + 1) * 512],
                            lhsT=ztown[k][:, t * 128 : (t + 1) * 128],
                            rhs=ztf[(blk, k)],
                            start=(k == 0),
                            stop=(k == KC - 1),
                        )
                # exp(2*G) in place in PSUM; rowsum over both halves via accum
                nc.scalar.activation(
                    pm, pm, mybir.ActivationFunctionType.Exp, scale=2.0,
                    accum_out=rs[:, t, bb : bb + 1],
                )
            for half in range(2):
                blk = 2 * bb + half
                # S1T row for this block: [B, 512]
                pst = ps_sm.tile([B, 512], F32, tag="sm", name="ps_s1t")
                for k in range(KC):
                    nc.tensor.matmul(
                        pst, lhsT=zbagT[:, k, :], rhs=ztf[(blk, k)],
                        start=(k == 0), stop=(k == KC - 1),
                    )
                nc.scalar.activation(
                    pst, pst, mybir.ActivationFunctionType.Exp, scale=2.0,
                    accum_out=cs[:, blk : blk + 1],
                )

        # ---- denominators + logs ----
        # denom_i [128, TPC]
        rsum = persist.tile([128, TPC], F32, name="rsum")
        nc.vector.reduce_sum(rsum, rs, axis=mybir.AxisListType.X)
        di = persist.tile([128, TPC], F32, name="di")
        nc.vector.tensor_scalar(
            out=di, in0=s1rs, scalar1=512.0, scalar2=-E2,
            op0=mybir.AluOpType.mult, op1=mybir.AluOpType.add,
        )
        nc.vector.tensor_add(di, di, rsum)
        ldi = persist.tile([128, TPC], F32, name="ldi")
        nc.scalar.activation(ldi, di, mybir.ActivationFunctionType.Ln)

        # denom_j [B, 1]
        csum = persist.tile([B, 1], F32, name="csum")
        nc.vector.reduce_sum(csum, cs, axis=mybir.AxisListType.X)
        dj = persist.tile([B, 1], F32, name="dj")
        nc.vector.tensor_scalar(
            out=dj, in0=rsbg, scalar1=512.0, scalar2=-E2,
            op0=mybir.AluOpType.mult, op1=mybir.AluOpType.add,
        )
        nc.vector.tensor_add(dj, dj, csum)
        ldj = persist.tile([B, 1], F32, name="ldj")
        nc.scalar.activation(ldj, dj, mybir.ActivationFunctionType.Ln)

        # ---- final combine: fin[:,0] = sum_t ldi - 4*sum_t pos; fin[0:8,1] = 512*ldj
        fin = persist.tile([128, 2], F32, name="fin")
        nc.vector.memset(fin, 0.0)
        vsum = persist.tile([128, 1], F32, name="vsum")
        nc.vector.reduce_sum(vsum, ldi, axis=mybir.AxisListType.X)
        posr = persist.tile([128, 1], F32, name="posr")
        nc.vector.reduce_sum(posr, pos, axis=mybir.AxisListType.X)
        nc.vector.tensor_scalar(
            out=posr, in0=posr, scalar1=-4.0, scalar2=None,
            op0=mybir.AluOpType.mult,
        )
        nc.vector.tensor_add(fin[:, 0:1], vsum, posr)
        nc.scalar.mul(fin[0:B, 1:2], ldj, 512.0)

        pfin = ps_sm.tile([1, 2], F32, tag="sm", name="ps_fin")
        nc.tensor.matmul(pfin, lhsT=ones, rhs=fin, start=True, stop=True)
        outt = persist.tile([1, 2], F32, name="outt")
        nc.vector.tensor_copy(outt, pfin)
        nc.sync.dma_start(out=out_d[:, :], in_=outt)

    return nc


def _get_nc():
    global _CACHED_NC
    if _CACHED_NC is None:
        nc = bacc.Bacc(
            "TRN2", target_bir_lowering=False, debug=False, num_devices=NCORES
        )
        nc = _build_kernel(nc)
        nc.compile()
        _CACHED_NC = nc
    return _CACHED_NC


def kernel(instance_emb: np.ndarray, bag_emb: np.ndarray) -> np.ndarray:
    global LAST_EXEC_TIME_NS
    Y = np.ascontiguousarray(
        np.asarray(instance_emb, dtype=np.float32).reshape(BS, D)
    )
    bg = np.ascontiguousarray(np.asarray(bag_emb, dtype=np.float32))

    in_maps = []
    for c in range(NCORES):
        oh = np.zeros((1, B), np.float32)
        oh[0, c] = 1.0
        in_maps.append(
            {
                "y_own": np.ascontiguousarray(Y[c * RPC : (c + 1) * RPC]),
                "bag": bg,
                "onehot": oh,
            }
        )

    nc = _get_nc()
    trace = os.environ.get("CL_KERNEL_TRACE", "0") == "1"
    tmpdir = os.environ.get("CL_KERNEL_TRACE_DIR") or None
    res = bass_utils.run_bass_kernel_spmd(
        nc, in_maps, core_ids=list(range(NCORES)), trace=trace, tmpdir=tmpdir
    )
    LAST_EXEC_TIME_NS = res.exec_time_ns

    total = 0.0
    for c in range(NCORES):
        total += float(res.results[c]["out"][0, 0])
    total += float(res.results[0]["out"][0, 1])
    return np.float32(total / (2 * BS))
